# revision 14
# baseline (speedup 1.0000x reference)
"""Trainium2 kernel for nn_DistanceRelativeBias.

Computes out[b,k,i,j] = g_k(||c_i - c_j||) where g_k(d) = b2[k] +
sum_h w2[h,k]*silu(w1[h]*d + b1[h]).

Strategy: the 16 head-functions g_k are scalar functions of the pairwise
distance d. On the host we refit them (near-exactly, residual ~1e-8) onto a
16-term silu basis  g_k(d) = sum_f Q[f,k]*silu(a_f*d + c_f)  chosen by OMP
over a candidate dictionary with a diversity constraint (keeps the TF32
matmul rounding amplification small).

Per core (8 cores: core = (batch b, half h) handles 512 query rows x 1024 keys):
  1. PE  (fp32):  d2[i,j] = -2*c_i.c_j + r_i + r_j via K=5 gram matmul
  2. DVE:         clamp d2 >= 0
  3. ACT:         d = sqrt(d2)  -> f32r tile
  4. PE  (f32r):  broadcast-matmul: selector SEL_v scatters 8 query rows
                  across 128 partitions (partition p=8f+g holds a_f * d of
                  row 8v+g), 8 pairs per moving column
  5. ACT:         phi = silu(. + c_f) with per-partition bias -> f32r
  6. PE  (f32r):  heads matmul, block-permuted Q [128x128]:
                  psum[16g+k] = sum_f Q[f,k] phi[8f+g]  (128 outputs/cycle)
  7. DVE:         psum -> sbuf copy
  8. DMA:         sbuf -> DRAM out[16,512,1024] (4KB contiguous rows)
"""
import numpy as np

B, N, D = 4, 1024, 3
HID, HEADS = 64, 16
NCORES = 8
IPC = N // 2            # i-rows per core (512)
NCHUNK = IPC // 128     # 4 chunks of 128 i-rows
NGRP = 16               # groups of 8 i-rows per chunk
NF = 16                 # basis size (features per pair)

# packed const tensor column layout
_SEL0 = 0                 # 16 selectors [128,128] -> cols [0, 2048)
_QM0 = 2048               # Qmat [128,128]        -> cols [2048, 2176)
_CV0 = 2176               # cvec [128,1]          -> col 2176
_RHS0 = 2177              # rhsD [5,1024]         -> cols [2177, 3201)
_LHS0 = 3201              # lhsD [5,512]          -> cols [3201, 3713)
_CW = 3713

_prog_cache = {}


def _silu(x):
    return x / (1.0 + np.exp(-x))


def _fit_basis(coords, w1, b1, w2, b2):
    """Fit g_k(d) ~= sum_f Q[f,k] silu(a_f d + c_f). Returns (a[16], c[16], Q[16,16])."""
    c64 = coords.astype(np.float64)
    w1 = w1.astype(np.float64).reshape(-1)
    b1 = b1.astype(np.float64)
    w2 = w2.astype(np.float64)
    b2 = b2.astype(np.float64)

    # distance distribution (exact, f64)
    qs = []
    dmax = 0.0
    for b in range(B):
        cb = c64[b]
        r = (cb * cb).sum(1)
        d2 = np.maximum(r[:, None] + r[None, :] - 2.0 * (cb @ cb.T), 0.0)
        d = np.sqrt(d2)
        dmax = max(dmax, d.max())
        qs.append(np.quantile(d.ravel(), np.linspace(0, 1, 1024)))
    grid = np.sort(np.concatenate([np.linspace(0, dmax * 1.02, 4096)] + qs))

    def g(d):
        return _silu(d[:, None] * w1 + b1) @ w2 + b2

    G = g(grid)

    cand = [(float(w1[h]), float(b1[h])) for h in range(HID)]
    for a in (-2, -1.5, -1.25, -1.0, -0.8, -0.6, -0.45, -0.3,
              0.3, 0.45, 0.6, 0.8, 1.0, 1.25, 1.5, 2.0):
        for c in np.linspace(-9, 9, 37):
            cand.append((a, float(c)))
    cand = np.array(cand)
    Phi = _silu(grid[:, None] * cand[:, 0] + cand[:, 1])
    nrm = np.linalg.norm(Phi, axis=0)

    sel = []
    for _ in range(NF):
        if sel:
            A = Phi[:, sel]
            Qq, _ = np.linalg.qr(A)
            R = G - Qq @ (Qq.T @ G)
            P = Phi - Qq @ (Qq.T @ Phi)
        else:
            R, P = G, Phi
        n = np.linalg.norm(P, axis=0)
        score = np.linalg.norm(P.T @ R, axis=1) / np.maximum(n, 1e-12)
        score[n < 0.02 * nrm] = -1.0   # diversity: skip near-dependent units
        for j in sel:
            score[j] = -1.0
        sel.append(int(np.argmax(score)))

    A = Phi[:, sel]
    s = np.linalg.norm(A, axis=0)
    An = A / s
    Q = np.linalg.solve(An.T @ An + 1e-8 * np.eye(NF), An.T @ G) / s[:, None]
    return cand[sel, 0].copy(), cand[sel, 1].copy(), Q


def _make_cst(coords_b, half, avec, cvec, Q):
    """Per-core packed const array [128, _CW] f32."""
    cst = np.zeros((128, _CW), dtype=np.float32)
    a_perm = np.repeat(avec, 8).astype(np.float32)   # a_perm[p] = a[p//8]
    c_perm = np.repeat(cvec, 8).astype(np.float32)

    # selector (t, s): superblock t covers chunk rows [32t, 32t+32); the
    # partition slot g takes row 32t + 4g + s.  Row-striping by 4 makes each
    # output partition's 4 j-runs DRAM-contiguous (16KB descriptors).
    p = np.arange(128)
    for t in range(4):
        for s in range(4):
            v = 4 * t + s
            S = np.zeros((128, 128), dtype=np.float32)
            S[32 * t + 4 * (p % 8) + s, p] = a_perm[p]
            cst[:, _SEL0 + 128 * v:_SEL0 + 128 * (v + 1)] = S

    # heads matmul weights; output partition m = 8k + g so that the 8 query
    # rows of one head are partition-adjacent -> 32KB contiguous DRAM runs
    Qm = np.zeros((128, 128), dtype=np.float32)
    for f in range(NF):
        for gslot in range(8):
            for k in range(HEADS):
                Qm[8 * f + gslot, 8 * k + gslot] = np.float32(Q[f, k])
    cst[:, _QM0:_QM0 + 128] = Qm
    cst[:, _CV0] = c_perm

    c32 = coords_b.astype(np.float64)
    r = (c32 * c32).sum(1)
    rhsD = np.zeros((5, N), dtype=np.float32)
    rhsD[0:3] = c32.T
    rhsD[3] = 1.0
    rhsD[4] = r
    cst[0:5, _RHS0:_RHS0 + N] = rhsD

    i0 = half * IPC
    ci = c32[i0:i0 + IPC]
    lhsD = np.zeros((5, IPC), dtype=np.float32)
    lhsD[0:3] = -2.0 * ci.T
    lhsD[3] = r[i0:i0 + IPC]
    lhsD[4] = 1.0
    cst[0:5, _LHS0:_LHS0 + IPC] = lhsD
    return cst


def _build_program():
    import concourse.bacc as bacc
    import concourse.mybir as mybir
    import concourse.tile as tile

    f32 = mybir.dt.float32
    f32r = mybir.dt.float32r
    AF = mybir.ActivationFunctionType

    nc = bacc.Bacc(num_devices=NCORES)
    CST = nc.declare_dram_parameter("cst", [128, _CW], f32, isOutput=False)
    OUT = nc.declare_dram_parameter("out", [HEADS, IPC, N], f32, isOutput=True)

    with tile.TileContext(nc) as tc:
        with (
            tc.tile_pool(name="const", bufs=1) as cp,
            tc.tile_pool(name="dtiles", bufs=NCHUNK) as dp,
            tc.tile_pool(name="wq", bufs=2) as wq,
            tc.tile_pool(name="wphi", bufs=4) as wphi,
            tc.tile_pool(name="wout", bufs=3) as wout,
            tc.tile_pool(name="psA", bufs=2, space="PSUM") as psA,
            tc.tile_pool(name="psB", bufs=1, space="PSUM") as psB,
        ):
            # ---- constants ----
            # small critical region (Qmat+cvec+rhsD+lhsD) first so the d2
            # matmuls start immediately; the 1MB selector block loads behind it
            cst = cp.tile([128, _CW], f32, tag="cst")
            nc.sync.dma_start(cst[:, _QM0:_CW], CST[:, _QM0:_CW])
            nc.scalar.dma_start(cst[:, _SEL0:_SEL0 + 2048], CST[:, _SEL0:_SEL0 + 2048])
            selr = cp.tile([128, 2048], f32r, tag="selr")
            nc.vector.tensor_copy(selr[:], cst[:, _SEL0:_SEL0 + 2048])
            qmr = cp.tile([128, 128], f32r, tag="qmr")
            nc.vector.tensor_copy(qmr[:], cst[:, _QM0:_QM0 + 128])
            cvec = cst[:, _CV0:_CV0 + 1]

            # ---- ACT table warmup (dependency-light) ----
            warm = cp.tile([128, 8], f32, tag="warm")
            nc.gpsimd.memset(warm[:], 0.0)
            warm2 = cp.tile([128, 8], f32, tag="warm2")
            nc.scalar.activation(warm2[:], warm[:], AF.Sqrt)

            # ---- phase 1: distances (all sqrts before any silu: the sqrt and
            # silu ACT table sets cannot coexist, so phase-separate) ----
            d_t = []
            for ic in range(NCHUNK):
                pd = psA.tile([128, N], f32, tag="ps")
                for hh in range(2):
                    nc.tensor.matmul(
                        pd[:, 512 * hh:512 * (hh + 1)],
                        cst[0:5, _LHS0 + 128 * ic:_LHS0 + 128 * (ic + 1)],
                        cst[0:5, _RHS0 + 512 * hh:_RHS0 + 512 * (hh + 1)],
                        start=True, stop=True)
                dsq = wq.tile([128, N], f32, tag="dsq")
                nc.vector.tensor_scalar(dsq[:], pd[:], 0.0, None, mybir.AluOpType.max)
                dt = dp.tile([128, N], f32r, tag="dt")
                nc.scalar.activation(dt[:], dsq[:], AF.Sqrt)
                d_t.append(dt)

            # re-arm the silu table; reads the last d tile so it stays ordered
            # after every sqrt on the ACT queue.
            nc.scalar.activation(warm2[:], d_t[-1][0:128, 0:8], AF.Silu)

            # ---- phase 2: broadcast -> silu -> heads -> store ----
            # per superblock (32 i-rows): 4 subgroups; heads outputs pair up in
            # one 4-bank psum tile (one wide DVE copy per pair), and the 4
            # subgroups pack one [128, 4096] sbuf tile = one 2MB DMA with 16KB
            # contiguous runs.
            ndma = 0
            for ic in range(NCHUNK):
                dt = d_t[ic]
                for t in range(4):
                    osb = wout.tile([128, 4 * N], f32, tag="osb")
                    for sp in range(2):           # subgroup pairs (0,1), (2,3)
                        phis = []
                        for s2 in range(2):
                            v = 4 * t + 2 * sp + s2
                            pb = psA.tile([128, N], f32, tag="ps")
                            for hh in range(2):
                                nc.tensor.matmul(
                                    pb[:, 512 * hh:512 * (hh + 1)],
                                    selr[:, 128 * v:128 * (v + 1)],
                                    dt[:, 512 * hh:512 * (hh + 1)],
                                    start=True, stop=True)
                            phi = wphi.tile([128, N], f32r, tag="phi")
                            nc.scalar.activation(phi[:], pb[:], AF.Silu,
                                                 bias=cvec, scale=1.0)
                            phis.append(phi)
                        po = psB.tile([128, 2 * N], f32, tag="po")
                        for s2 in range(2):
                            for hh in range(2):
                                nc.tensor.matmul(
                                    po[:, N * s2 + 512 * hh:N * s2 + 512 * (hh + 1)],
                                    qmr[:],
                                    phis[s2][:, 512 * hh:512 * (hh + 1)],
                                    start=True, stop=True)
                        nc.vector.tensor_copy(
                            osb[:, 2 * N * sp:2 * N * (sp + 1)], po[:])
                    i0 = 128 * ic + 32 * t
                    eng = (nc.sync, nc.scalar, nc.gpsimd)[ndma % 3]
                    ndma += 1
                    eng.dma_start(
                        OUT[:, i0:i0 + 32, :].rearrange(
                            "k (g four) j -> k g (four j)", four=4),
                        osb[:])
    nc.compile()
    return nc


def _run(coords, w1, b1, w2, b2, trace=False):
    from concourse.bass_utils import run_bass_kernel_spmd

    avec, cvec, Q = _fit_basis(coords, w1, b1, w2, b2)
    if "nc" not in _prog_cache:
        _prog_cache["nc"] = _build_program()
    nc = _prog_cache["nc"]

    in_maps = []
    for core in range(NCORES):
        b, h = divmod(core, 2)
        in_maps.append({"cst": _make_cst(np.asarray(coords)[b], h, avec, cvec, Q)})

    res = run_bass_kernel_spmd(nc, in_maps, list(range(NCORES)), trace=trace)

    out = np.empty((B, HEADS, N, N), dtype=np.float32)
    for core in range(NCORES):
        b, h = divmod(core, 2)
        out[b, :, h * IPC:(h + 1) * IPC, :] = res.results[core]["out"]
    return out, res


def kernel(coords, w1, b1, w2, b2):
    out, _ = _run(coords, w1, b1, w2, b2, trace=False)
    return out


# revision 16
# speedup vs baseline: 1.0953x; 1.0953x over previous
"""Trainium2 kernel for nn_DistanceRelativeBias.

Computes out[b,k,i,j] = g_k(||c_i - c_j||) where g_k(d) = b2[k] +
sum_h w2[h,k]*silu(w1[h]*d + b1[h]).

Strategy: the 16 head-functions g_k are scalar functions of the pairwise
distance d. On the host we refit them (near-exactly, residual ~1e-8) onto a
16-term silu basis  g_k(d) = sum_f Q[f,k]*silu(a_f*d + c_f)  chosen by OMP
over a candidate dictionary with a diversity constraint (keeps the TF32
matmul rounding amplification small).

Per core (8 cores: core = (batch b, half h) handles 512 query rows x 1024 keys):
  1. PE  (fp32):  d2[i,j] = -2*c_i.c_j + r_i + r_j via K=5 gram matmul
  2. DVE:         clamp d2 >= 0
  3. ACT:         d = sqrt(d2)  -> f32r tile
  4. PE  (f32r):  broadcast-matmul: selector SEL_v scatters 8 query rows
                  across 128 partitions (partition p=8f+g holds a_f * d of
                  row 8v+g), 8 pairs per moving column
  5. ACT:         phi = silu(. + c_f) with per-partition bias -> f32r
  6. PE  (f32r):  heads matmul, block-permuted Q [128x128]:
                  psum[16g+k] = sum_f Q[f,k] phi[8f+g]  (128 outputs/cycle)
  7. DVE:         psum -> sbuf copy
  8. DMA:         sbuf -> DRAM out[16,512,1024] (4KB contiguous rows)
"""
import numpy as np

B, N, D = 4, 1024, 3
HID, HEADS = 64, 16
NCORES = 8
IPC = N // 2            # i-rows per core (512)
NCHUNK = IPC // 128     # 4 chunks of 128 i-rows
NGRP = 16               # groups of 8 i-rows per chunk
NF = 16                 # basis size (features per pair)

# packed const tensor column layout
_SEL0 = 0                 # 16 selectors [128,128] -> cols [0, 2048)
_QM0 = 2048               # Qmat [128,128]        -> cols [2048, 2176)
_CV0 = 2176               # cvec [128,1]          -> col 2176
_RHS0 = 2177              # rhsD [5,1024]         -> cols [2177, 3201)
_LHS0 = 3201              # lhsD [5,512]          -> cols [3201, 3713)
_CW = 3713

_prog_cache = {}


def _silu(x):
    return x / (1.0 + np.exp(-x))


def _fit_basis(coords, w1, b1, w2, b2):
    """Fit g_k(d) ~= sum_f Q[f,k] silu(a_f d + c_f). Returns (a[16], c[16], Q[16,16])."""
    c64 = coords.astype(np.float64)
    w1 = w1.astype(np.float64).reshape(-1)
    b1 = b1.astype(np.float64)
    w2 = w2.astype(np.float64)
    b2 = b2.astype(np.float64)

    # distance distribution (exact, f64)
    qs = []
    dmax = 0.0
    for b in range(B):
        cb = c64[b]
        r = (cb * cb).sum(1)
        d2 = np.maximum(r[:, None] + r[None, :] - 2.0 * (cb @ cb.T), 0.0)
        d = np.sqrt(d2)
        dmax = max(dmax, d.max())
        qs.append(np.quantile(d.ravel(), np.linspace(0, 1, 1024)))
    grid = np.sort(np.concatenate([np.linspace(0, dmax * 1.02, 4096)] + qs))

    def g(d):
        return _silu(d[:, None] * w1 + b1) @ w2 + b2

    G = g(grid)

    cand = [(float(w1[h]), float(b1[h])) for h in range(HID)]
    for a in (-2, -1.5, -1.25, -1.0, -0.8, -0.6, -0.45, -0.3,
              0.3, 0.45, 0.6, 0.8, 1.0, 1.25, 1.5, 2.0):
        for c in np.linspace(-9, 9, 37):
            cand.append((a, float(c)))
    cand = np.array(cand)
    Phi = _silu(grid[:, None] * cand[:, 0] + cand[:, 1])
    nrm = np.linalg.norm(Phi, axis=0)

    sel = []
    for _ in range(NF):
        if sel:
            A = Phi[:, sel]
            Qq, _ = np.linalg.qr(A)
            R = G - Qq @ (Qq.T @ G)
            P = Phi - Qq @ (Qq.T @ Phi)
        else:
            R, P = G, Phi
        n = np.linalg.norm(P, axis=0)
        score = np.linalg.norm(P.T @ R, axis=1) / np.maximum(n, 1e-12)
        score[n < 0.02 * nrm] = -1.0   # diversity: skip near-dependent units
        for j in sel:
            score[j] = -1.0
        sel.append(int(np.argmax(score)))

    A = Phi[:, sel]
    s = np.linalg.norm(A, axis=0)
    An = A / s
    Q = np.linalg.solve(An.T @ An + 1e-8 * np.eye(NF), An.T @ G) / s[:, None]
    return cand[sel, 0].copy(), cand[sel, 1].copy(), Q


def _make_cst(coords_b, half, avec, cvec, Q):
    """Per-core packed const array [128, _CW] f32."""
    cst = np.zeros((128, _CW), dtype=np.float32)
    a_perm = np.repeat(avec, 8).astype(np.float32)   # a_perm[p] = a[p//8]
    c_perm = np.repeat(cvec, 8).astype(np.float32)

    # selector (t, s): superblock t covers chunk rows [32t, 32t+32); the
    # partition slot g takes row 32t + 4g + s.  Row-striping by 4 makes each
    # output partition's 4 j-runs DRAM-contiguous (16KB descriptors).
    p = np.arange(128)
    for t in range(4):
        for s in range(4):
            v = 4 * t + s
            S = np.zeros((128, 128), dtype=np.float32)
            S[32 * t + 4 * (p % 8) + s, p] = a_perm[p]
            cst[:, _SEL0 + 128 * v:_SEL0 + 128 * (v + 1)] = S

    # heads matmul weights; output partition m = 8k + g so that the 8 query
    # rows of one head are partition-adjacent -> 32KB contiguous DRAM runs
    Qm = np.zeros((128, 128), dtype=np.float32)
    for f in range(NF):
        for gslot in range(8):
            for k in range(HEADS):
                Qm[8 * f + gslot, 8 * k + gslot] = np.float32(Q[f, k])
    cst[:, _QM0:_QM0 + 128] = Qm
    cst[:, _CV0] = c_perm

    c32 = coords_b.astype(np.float64)
    r = (c32 * c32).sum(1)
    rhsD = np.zeros((5, N), dtype=np.float32)
    rhsD[0:3] = c32.T
    rhsD[3] = 1.0
    rhsD[4] = r
    cst[0:5, _RHS0:_RHS0 + N] = rhsD

    i0 = half * IPC
    ci = c32[i0:i0 + IPC]
    lhsD = np.zeros((5, IPC), dtype=np.float32)
    lhsD[0:3] = -2.0 * ci.T
    lhsD[3] = r[i0:i0 + IPC]
    lhsD[4] = 1.0
    cst[0:5, _LHS0:_LHS0 + IPC] = lhsD
    return cst


def _build_program():
    import concourse.bacc as bacc
    import concourse.mybir as mybir
    import concourse.tile as tile

    f32 = mybir.dt.float32
    f32r = mybir.dt.float32r
    AF = mybir.ActivationFunctionType

    nc = bacc.Bacc(num_devices=NCORES)
    CST = nc.declare_dram_parameter("cst", [128, _CW], f32, isOutput=False)
    OUT = nc.declare_dram_parameter("out", [HEADS, IPC, N], f32, isOutput=True)

    with tile.TileContext(nc) as tc:
        with (
            tc.tile_pool(name="const", bufs=1) as cp,
            tc.tile_pool(name="dtiles", bufs=NCHUNK) as dp,
            tc.tile_pool(name="wq", bufs=2) as wq,
            tc.tile_pool(name="wphi", bufs=4) as wphi,
            tc.tile_pool(name="wout", bufs=3) as wout,
            tc.tile_pool(name="psA", bufs=2, space="PSUM") as psA,
            tc.tile_pool(name="psB", bufs=2, space="PSUM") as psB,
        ):
            # ---- constants ----
            # small critical region (Qmat+cvec+rhsD+lhsD) first so the d2
            # matmuls start immediately; the 1MB selector block loads behind it
            cst = cp.tile([128, _CW], f32, tag="cst")
            nc.sync.dma_start(cst[:, _QM0:_CW], CST[:, _QM0:_CW])
            nc.scalar.dma_start(cst[:, _SEL0:_SEL0 + 2048], CST[:, _SEL0:_SEL0 + 2048])
            selr = cp.tile([128, 2048], f32r, tag="selr")
            nc.vector.tensor_copy(selr[:], cst[:, _SEL0:_SEL0 + 2048])
            qmr = cp.tile([128, 128], f32r, tag="qmr")
            nc.vector.tensor_copy(qmr[:], cst[:, _QM0:_QM0 + 128])
            cvec = cst[:, _CV0:_CV0 + 1]

            # ---- ACT table warmup (dependency-light) ----
            warm = cp.tile([128, 8], f32, tag="warm")
            nc.gpsimd.memset(warm[:], 0.0)
            warm2 = cp.tile([128, 8], f32, tag="warm2")
            nc.scalar.activation(warm2[:], warm[:], AF.Sqrt)

            # ---- phase 1: distances (all sqrts before any silu: the sqrt and
            # silu ACT table sets cannot coexist, so phase-separate) ----
            d_t = []
            for ic in range(NCHUNK):
                pd = psA.tile([128, N], f32, tag="ps")
                for hh in range(2):
                    nc.tensor.matmul(
                        pd[:, 512 * hh:512 * (hh + 1)],
                        cst[0:5, _LHS0 + 128 * ic:_LHS0 + 128 * (ic + 1)],
                        cst[0:5, _RHS0 + 512 * hh:_RHS0 + 512 * (hh + 1)],
                        start=True, stop=True)
                dsq = wq.tile([128, N], f32, tag="dsq")
                nc.vector.tensor_scalar(dsq[:], pd[:], 0.0, None, mybir.AluOpType.max)
                dt = dp.tile([128, N], f32r, tag="dt")
                nc.scalar.activation(dt[:], dsq[:], AF.Sqrt)
                d_t.append(dt)

            # re-arm the silu table; reads the last d tile so it stays ordered
            # after every sqrt on the ACT queue.
            nc.scalar.activation(warm2[:], d_t[-1][0:128, 0:8], AF.Silu)

            # ---- phase 2: broadcast -> silu -> heads -> store ----
            # per superblock (32 i-rows): 4 subgroups; heads outputs pair up in
            # one 4-bank psum tile (one wide DVE copy per pair), and the 4
            # subgroups pack one [128, 4096] sbuf tile = one 2MB DMA with 16KB
            # contiguous runs.
            ndma = 0
            for ic in range(NCHUNK):
                dt = d_t[ic]
                for t in range(4):
                    osb = wout.tile([128, 4 * N], f32, tag="osb")
                    for s in range(4):
                        v = 4 * t + s
                        pb = psA.tile([128, N], f32, tag="ps")
                        for hh in range(2):
                            nc.tensor.matmul(
                                pb[:, 512 * hh:512 * (hh + 1)],
                                selr[:, 128 * v:128 * (v + 1)],
                                dt[:, 512 * hh:512 * (hh + 1)],
                                start=True, stop=True)
                        phi = wphi.tile([128, N], f32r, tag="phi")
                        nc.scalar.activation(phi[:], pb[:], AF.Silu,
                                             bias=cvec, scale=1.0)
                        po = psB.tile([128, N], f32, tag="po")
                        for hh in range(2):
                            nc.tensor.matmul(
                                po[:, 512 * hh:512 * (hh + 1)],
                                qmr[:],
                                phi[:, 512 * hh:512 * (hh + 1)],
                                start=True, stop=True)
                        nc.vector.tensor_copy(
                            osb[:, N * s:N * (s + 1)], po[:])
                    i0 = 128 * ic + 32 * t
                    eng = (nc.sync, nc.scalar, nc.gpsimd)[ndma % 3]
                    ndma += 1
                    eng.dma_start(
                        OUT[:, i0:i0 + 32, :].rearrange(
                            "k (g four) j -> k g (four j)", four=4),
                        osb[:])
    nc.compile()
    return nc


def _run(coords, w1, b1, w2, b2, trace=False):
    from concourse.bass_utils import run_bass_kernel_spmd

    avec, cvec, Q = _fit_basis(coords, w1, b1, w2, b2)
    if "nc" not in _prog_cache:
        _prog_cache["nc"] = _build_program()
    nc = _prog_cache["nc"]

    in_maps = []
    for core in range(NCORES):
        b, h = divmod(core, 2)
        in_maps.append({"cst": _make_cst(np.asarray(coords)[b], h, avec, cvec, Q)})

    res = run_bass_kernel_spmd(nc, in_maps, list(range(NCORES)), trace=trace)

    out = np.empty((B, HEADS, N, N), dtype=np.float32)
    for core in range(NCORES):
        b, h = divmod(core, 2)
        out[b, :, h * IPC:(h + 1) * IPC, :] = res.results[core]["out"]
    return out, res


def kernel(coords, w1, b1, w2, b2):
    out, _ = _run(coords, w1, b1, w2, b2, trace=False)
    return out
